# revision 35
# baseline (speedup 1.0000x reference)
"""Trainium2 Bass kernel for nn_BlankEmbedding (embedding gather + blank-run scan).

Math: the reference computes e = emb_table[x], then runs 8 iterations of
    pos = shift_right(pos); acc = shift_right(acc); out = out + acc; acc = out*pos
starting from pos = is_preblank.  Unrolling, out[i] = sum_d C[i,d] * e[i-d]
with banded integer coefficients C that depend only on x.  Rows with any
C[i,d>0] != 0 are rare (~1/16, grouped <=128 per output half), and their
distinct band sources {i-d} number at most one 128-row tile per group.

The whole kernel runs in bf16 (the harness gate is rel_err < 2e-2; bf16
round-off lands ~5e-3), which halves every DMA byte moved.

  per core (2048 of the 16384 rows, data-parallel over B*S):
    1. dma_gather the core's embedding rows from a deduplicated bf16 table
       (HBM->SBUF, chunks [512,512,512,384,128], each chunk with its own
       SBUF buffer), writing each chunk out with a strided DMA on the sync
       HWDGE ring.
    2. per affected-row group: ONE 128-row dma_gather (queues 1/2) fetches
       the group's distinct band-source rows E (dead slots index a zero row
       appended to the table, so no memsets and no NaN risk); the TENSOR
       engine computes the whole delta as one PSUM matmul W^T @ E, where
       the host-built W[k, m] = C[row_m, d] iff src_k == row_m - d; DVE
       copies PSUM -> SBUF bf16; a dma_scatter_add on queue 3 applies it
       onto the already-written output rows (the reg-loaded count skips
       partitions past the group's row count).

Host side only computes index lists / W matrices from x ([B,S] int ops) and
reassembles the 8 per-core outputs (upcasting bf16 -> f32).
"""

import numpy as np

B, S, D = 4, 4096, 2048
N_CORES = 8
RPC = (B * S) // N_CORES          # rows per core = 2048
CHUNK_SIZES = [256] * 8
N_CHUNKS = len(CHUNK_SIZES)
CHUNK_OFF = [sum(CHUNK_SIZES[:i]) for i in range(N_CHUNKS + 1)]
GPPS = [cs // 128 for cs in CHUNK_SIZES]  # rows per partition per chunk
REGIONS = [(0, 4), (4, 7), (7, 8)]        # chunk spans sharing one scatter
CPCS = [cs // 16 for cs in CHUNK_SIZES]   # idx columns per chunk
CPC_OFF = [sum(CPCS[:i]) for i in range(N_CHUNKS + 1)]
NBLANK_IDS = 16
N_ITER = 8
BAND = N_ITER + 1                 # out[i] depends on e[i-8..i]
PSW = 512                         # psum bank width (f32)
NPC = D // PSW                    # psum column chunks per row


def _cdiv(a, b):
    return (a + b - 1) // b


def _compute_coeffs(x):
    """C[b, s, d] for d=0..8 (float64 holds small ints exactly)."""
    b, s = x.shape
    blank = ((x >= 0) & (x < NBLANK_IDS)).astype(np.float64)
    shift_r = lambda t: np.concatenate([np.zeros_like(t[:, :1]), t[:, :-1]], axis=1)
    first = np.maximum(blank - shift_r(blank), 0.0)
    m = np.concatenate([first[:, 1:], np.zeros_like(first[:, :1])], axis=1)
    C = np.zeros((b, s, BAND))
    C[:, :, 0] = 1.0
    for k in range(1, N_ITER + 1):
        m_k = np.zeros_like(m)
        m_k[:, k:] = m[:, :-k]
        Cs = np.zeros_like(C)
        Cs[:, 1:, 1:] = C[:, :-1, :-1]
        C = C + m_k[:, :, None] * Cs
    return C


def _wrap16(vals, ncols):
    """Wrap a 1-D index list into the [128, ncols] int16 layout the SWDGE
    gather/scatter ucode expects: slot j at [j % 16, j // 16], replicated
    across the eight 16-partition Q7 core groups."""
    blk = np.zeros((16, ncols), dtype=np.int16)
    v = np.asarray(vals, dtype=np.int16)
    for j in range(len(v)):
        blk[j % 16, j // 16] = v[j]
    return np.tile(blk, (8, 1))


def _prepare(x_np):
    """All host-side index/W prep. Returns per-core arrays + meta.

    The device table is emb[uniq] with one extra all-zero row at index NV:
    dead band-source slots index it so they contribute exact zeros.
    """
    uniq, inv = np.unique(x_np, return_inverse=True)
    ridx = inv.reshape(x_np.shape).astype(np.int64)
    NV = len(uniq)
    assert NV + 1 <= 32767, "int16 gather index overflow"
    ZROW = NV                                         # the appended zero row

    C = _compute_coeffs(x_np)
    aff = (C[:, :, 1:] != 0).any(axis=2)              # [B,S]

    cores = []
    for c in range(N_CORES):
        b, h = c // 2, c % 2
        s0 = h * RPC
        midx = np.zeros((128, CPC_OFF[-1]), dtype=np.int16)
        for ch in range(N_CHUNKS):
            cs, gpp = CHUNK_SIZES[ch], GPPS[ch]
            slots = np.empty(cs, dtype=np.int64)
            for j in range(cs):
                l = (j % 128) * gpp + (j // 128) + CHUNK_OFF[ch]
                slots[j] = ridx[b, s0 + l]
            midx[:, CPC_OFF[ch]:CPC_OFF[ch + 1]] = _wrap16(slots, CPCS[ch])

        # affected rows per region (region = run of chunks; the last region
        # is the final chunk alone so the tail scatter is tiny), split
        # further if rows would need more than one 128-row source tile
        rows_all = np.nonzero(aff[b, s0:s0 + RPC])[0]
        Cc = C[b, s0:s0 + RPC]                        # [RPC, 9]
        halves = []
        for lo, hi in REGIONS:
            r0, r1 = CHUNK_OFF[lo], CHUNK_OFF[hi]
            rh = rows_all[(rows_all >= r0) & (rows_all < r1)]
            parts = []
            cur, srcs = [], set()
            for r in rh:
                new = {int(r) - d for d in range(1, N_ITER + 1) if Cc[r, d] != 0}
                if len(cur) >= 128 or len(srcs | new) > 128:
                    parts.append(np.array(cur, dtype=np.int64))
                    cur, srcs = [], set()
                cur.append(int(r))
                srcs |= new
            if cur:
                parts.append(np.array(cur, dtype=np.int64))
            halves.append(parts)
        cores.append(dict(b=b, s0=s0, halves=halves, Cc=Cc, midx=midx))

    # group g of region h waits for the writebacks covering that region
    H = [max(len(co["halves"][h]) for co in cores) for h in range(len(REGIONS))]
    G = sum(H)
    meta = dict(NV=NV, G=G, wait_chunks=[])
    if G == 0:
        for co in cores:
            co.update(bidx=None, sidx=None, wmat=None, cnts=None)
        return uniq, cores, meta
    group_defs = []   # (region, index_within_region)
    for h in range(len(REGIONS)):
        for k in range(H[h]):
            group_defs.append((h, k))
            meta["wait_chunks"].append(list(range(*REGIONS[h])))
    for co in cores:
        co["rows_g"] = [co["halves"][h][k] if k < len(co["halves"][h])
                        else np.empty(0, dtype=np.int64)
                        for h, k in group_defs]

    for co in cores:
        b, s0, Cc = co["b"], co["s0"], co["Cc"]
        bidx = np.zeros((128, G * 8), dtype=np.int16)
        sidx = np.zeros((128, G * 8), dtype=np.int16)
        wmat = np.zeros((128, G * 128), dtype=np.float32)
        cnts = np.zeros((1, G), dtype=np.int32)       # scatter counts
        for g in range(G):
            rg = co["rows_g"][g]
            cnts[0, g] = max(len(rg), 1)
            srcs = sorted({int(r) - d for r in rg
                           for d in range(1, N_ITER + 1) if Cc[r, d] != 0})
            assert len(srcs) <= 128
            src_slot = {s: k for k, s in enumerate(srcs)}
            vals = np.full(128, ZROW, dtype=np.int64)
            for s, k in src_slot.items():
                vals[k] = ridx[b, s0 + s]
            bidx[:, g * 8:(g + 1) * 8] = _wrap16(vals, 8)
            for m, r in enumerate(rg):
                for d in range(1, N_ITER + 1):
                    if Cc[r, d] != 0:
                        wmat[src_slot[int(r) - d], g * 128 + m] = Cc[r, d]

            tgts = np.full(128, -1, dtype=np.int64)
            if len(rg):
                tgts[:len(rg)] = rg
            else:
                tgts[0] = 0   # adds an exact 0 to row 0
            sidx[:, g * 8:(g + 1) * 8] = _wrap16(tgts, 8)
        co.update(bidx=bidx, sidx=sidx, wmat=wmat, cnts=cnts)
    return uniq, cores, meta


def _build_program(NV, G, wait_chunks):
    import concourse.bacc as bacc
    import concourse.mybir as mybir
    from concourse.library_config import mlp

    f32, i16, i32 = mybir.dt.float32, mybir.dt.int16, mybir.dt.int32
    bf16 = mybir.dt.bfloat16

    nc = bacc.Bacc("TRN2", target_bir_lowering=False, debug=False,
                   enable_asserts=False, num_devices=N_CORES,
                   num_swdge_queues=4, dynamic_dma_scratch_size=65536)
    table = nc.dram_tensor("table", [NV + 1, D], bf16, kind="ExternalInput")
    midx_d = nc.dram_tensor("midx", [128, CPC_OFF[-1]], i16, kind="ExternalInput")
    out_d = nc.dram_tensor("out", [RPC, D], bf16, kind="ExternalOutput")
    if G:
        bidx_d = nc.dram_tensor("bidx", [128, G * 8], i16, kind="ExternalInput")
        sidx_d = nc.dram_tensor("sidx", [128, G * 8], i16, kind="ExternalInput")
        wmat_d = nc.dram_tensor("wmat", [128, G * 128], bf16, kind="ExternalInput")
        cnts_d = nc.dram_tensor("cnts", [1, G], i32, kind="ExternalInput")

    from contextlib import ExitStack
    with ExitStack() as st:
        # every chunk gets its own buffer (bf16 halves SBUF): no reuse waits
        mbuf = [st.enter_context(nc.sbuf_tensor(f"mbuf{i}", [128, GPPS[i], D], bf16))
                for i in range(N_CHUNKS)]
        midx_s = st.enter_context(nc.sbuf_tensor("midx_s", [128, CPC_OFF[-1]], i16))
        m_sem = st.enter_context(nc.semaphore("m_sem"))
        g_sems = [st.enter_context(nc.semaphore(f"g_sem{c}")) for c in range(N_CHUNKS)]
        w_sems = [st.enter_context(nc.semaphore(f"w_sem{c}")) for c in range(N_CHUNKS)]
        if G:
            etile = [st.enter_context(nc.sbuf_tensor(f"etile{g}", [128, 1, D], bf16))
                     for g in range(G)]
            deltas = [st.enter_context(
                nc.sbuf_tensor(f"delta{g}", [128, 1, D], bf16))
                for g in range(G)]
            npsets = min(G, 2)
            psum = [[st.enter_context(
                nc.psum_tensor(f"ps{e}_{c}", [128, PSW], f32))
                for c in range(NPC)] for e in range(npsets)]
            bidx_s = st.enter_context(nc.sbuf_tensor("bidx_s", [128, G * 8], i16))
            sidx_s = st.enter_context(nc.sbuf_tensor("sidx_s", [128, G * 8], i16))
            wmat_s = st.enter_context(nc.sbuf_tensor("wmat_s", [128, G * 128], bf16))
            cnts_s = st.enter_context(nc.sbuf_tensor("cnts_s", [1, G], i32))
            nregs = [st.enter_context(nc.gpsimd.register(f"nreg{g}"))
                     for g in range(G)]
            bi_sem = st.enter_context(nc.semaphore("bi_sem"))
            si_sem = st.enter_context(nc.semaphore("si_sem"))
            cd_sem = st.enter_context(nc.semaphore("cd_sem"))
            b_sems = [st.enter_context(nc.semaphore(f"b_sem{g}")) for g in range(G)]
            pe_sems = [st.enter_context(nc.semaphore(f"pe_sem{g}")) for g in range(G)]
            d_sems = [st.enter_context(nc.semaphore(f"d_sem{g}")) for g in range(G)]
            s_sem = st.enter_context(nc.semaphore("s_sem"))
            p_sem = st.enter_context(nc.semaphore("p_sem"))
        block = st.enter_context(nc.Block(no_gpsimd_drain=True))

        def writeback(eng, ch):
            eng.wait_ge(g_sems[ch], 16)
            dst = out_d[CHUNK_OFF[ch]:CHUNK_OFF[ch + 1], :].rearrange(
                "(p g) e -> p g e", g=GPPS[ch])
            eng.dma_start(dst, mbuf[ch][:, :, :]).then_inc(w_sems[ch], 16)

        @block.sync
        def _(sync):
            sync.dma_start(midx_s[:, :], midx_d[:, :]).then_inc(m_sem, 16)
            for ch in range(0, N_CHUNKS, 2):
                writeback(sync, ch)

        @block.scalar
        def _(scalar):
            if G:
                scalar.dma_start(bidx_s[:, :], bidx_d[:, :]).then_inc(bi_sem, 16)
                scalar.dma_start(cnts_s[:, :], cnts_d[:, :]).then_inc(bi_sem, 16)
                scalar.dma_start(wmat_s[:, :], wmat_d[:, :]).then_inc(cd_sem, 16)
                scalar.dma_start(sidx_s[:, :], sidx_d[:, :]).then_inc(si_sem, 16)
            for ch in range(1, N_CHUNKS, 2):
                writeback(scalar, ch)

        @block.gpsimd
        def _(gp):
            gp.load_library(mlp)
            gp.wait_ge(m_sem, 16)

            def main_gather(ch):
                cs = CHUNK_SIZES[ch]
                gp.dma_gather(mbuf[ch][:, :, :], table[:, :],
                              midx_s[:, CPC_OFF[ch]:CPC_OFF[ch + 1]],
                              cs, cs, D,
                              single_packet=False).then_inc(g_sems[ch], 16)

            def band_gather(g):
                gp.dma_gather(etile[g][:, :, :], table[:, :],
                              bidx_s[:, g * 8:(g + 1) * 8],
                              128, 128, D,
                              single_packet=False,
                              queue_num=1 + g % 2).then_inc(b_sems[g], 16)

            main_gather(0)
            main_gather(1)
            if G:
                gp.wait_ge(bi_sem, 32)
                for g in range(G):
                    band_gather(g)
            for ch in range(2, N_CHUNKS):
                main_gather(ch)
            if G:
                # scatters: descriptors prepped early (reads sidx + count
                # regs now), fired by trigger_dma once the writebacks and
                # deltas land - keeps desc-gen off the critical tail
                gp.wait_ge(si_sem, 16)
                for g in range(G):
                    gp.reg_load(nregs[g], cnts_s[0:1, g:g + 1])
                    gp.dma_scatter_add(out_d[:, :], deltas[g][:, :, :],
                                       sidx_s[:, g * 8:(g + 1) * 8],
                                       128, nregs[g], D,
                                       single_packet=False,
                                       queue_num=3, prepare_only=True,
                                       sem=s_sem).then_inc(p_sem, 1)
                for g in range(G):
                    gp.wait_ge(p_sem, g + 1)
                    for c in wait_chunks[g]:
                        gp.wait_ge(w_sems[c], 16)
                    gp.wait_ge(d_sems[g], 1)
                    gp.trigger_dma(count=1, queue_num=3)
                gp.wait_ge(s_sem, 16 * G)

        @block.tensor
        def _(pe):
            if not G:
                return
            pe.wait_ge(cd_sem, 16)
            for g in range(G):
                e = g % 2
                if g >= 2:
                    pe.wait_ge(d_sems[g - 2], 1)   # psum set free again
                pe.wait_ge(b_sems[g], 16)
                for c in range(NPC):
                    ins = pe.matmul(
                        psum[e][c][:, :],
                        wmat_s[:, g * 128:(g + 1) * 128],
                        etile[g][:, 0, c * PSW:(c + 1) * PSW],
                        start=True, stop=True)
                ins.then_inc(pe_sems[g], 1)

        @block.vector
        def _(v):
            if not G:
                return
            for g in range(G):
                v.wait_ge(pe_sems[g], 1)
                for c in range(NPC):
                    ins = v.tensor_copy(deltas[g][:, 0, c * PSW:(c + 1) * PSW],
                                        psum[g % 2][c][:, :])
                ins.then_inc(d_sems[g], 1)

    nc.compile()
    return nc


_CACHE = {}
_LAST_RESULT = None


def kernel(x, emb_table):
    global _LAST_RESULT
    import ml_dtypes
    from concourse.bass_utils import run_bass_kernel_spmd

    x_np = np.asarray(x)
    emb_np = np.asarray(emb_table, dtype=np.float32)
    uniq, cores, meta = _prepare(x_np)
    table_sl = np.zeros((meta["NV"] + 1, D), dtype=ml_dtypes.bfloat16)
    table_sl[:meta["NV"]] = emb_np[uniq].astype(ml_dtypes.bfloat16)

    key = (meta["NV"], meta["G"],
           tuple(map(tuple, meta["wait_chunks"])))
    if key not in _CACHE:
        _CACHE[key] = _build_program(meta["NV"], meta["G"],
                                     meta["wait_chunks"])
    nc = _CACHE[key]

    in_maps = []
    for co in cores:
        m = {"table": table_sl, "midx": co["midx"]}
        if meta["G"]:
            m.update(bidx=co["bidx"], sidx=co["sidx"],
                     wmat=co["wmat"].astype(ml_dtypes.bfloat16),
                     cnts=co["cnts"])
        in_maps.append(m)

    res = run_bass_kernel_spmd(nc, in_maps, core_ids=list(range(N_CORES)))
    _LAST_RESULT = res
    full = np.empty((B, S, D), dtype=np.float32)
    for c in range(N_CORES):
        b, h = c // 2, c % 2
        full[b, h * RPC:(h + 1) * RPC, :] = res.results[c]["out"].astype(np.float32)
    return full


# revision 36
# speedup vs baseline: 1.0653x; 1.0653x over previous
"""Trainium2 Bass kernel for nn_BlankEmbedding (embedding gather + blank-run scan).

Math: the reference computes e = emb_table[x], then runs 8 iterations of
    pos = shift_right(pos); acc = shift_right(acc); out = out + acc; acc = out*pos
starting from pos = is_preblank.  Unrolling, out[i] = sum_d C[i,d] * e[i-d]
with banded integer coefficients C that depend only on x.  Rows with any
C[i,d>0] != 0 are rare (~1/16, grouped <=128 per output half), and their
distinct band sources {i-d} number at most one 128-row tile per group.

The whole kernel runs in bf16 (the harness gate is rel_err < 2e-2; bf16
round-off lands ~5e-3), which halves every DMA byte moved.

  per core (2048 of the 16384 rows, data-parallel over B*S):
    1. dma_gather the core's embedding rows from a deduplicated bf16 table
       (HBM->SBUF, chunks [512,512,512,384,128], each chunk with its own
       SBUF buffer), writing each chunk out with a strided DMA on the sync
       HWDGE ring.
    2. per affected-row group: ONE 128-row dma_gather (queues 1/2) fetches
       the group's distinct band-source rows E (dead slots index a zero row
       appended to the table, so no memsets and no NaN risk); the TENSOR
       engine computes the whole delta as one PSUM matmul W^T @ E, where
       the host-built W[k, m] = C[row_m, d] iff src_k == row_m - d; DVE
       copies PSUM -> SBUF bf16; a dma_scatter_add on queue 3 applies it
       onto the already-written output rows (the reg-loaded count skips
       partitions past the group's row count).

Host side only computes index lists / W matrices from x ([B,S] int ops) and
reassembles the 8 per-core outputs (upcasting bf16 -> f32).
"""

import numpy as np

B, S, D = 4, 4096, 2048
N_CORES = 8
RPC = (B * S) // N_CORES          # rows per core = 2048
CHUNK_SIZES = [256] * 8
N_CHUNKS = len(CHUNK_SIZES)
CHUNK_OFF = [sum(CHUNK_SIZES[:i]) for i in range(N_CHUNKS + 1)]
GPPS = [cs // 128 for cs in CHUNK_SIZES]  # rows per partition per chunk
REGIONS = [(0, 4), (4, 8)]                # chunk spans sharing one scatter
CPCS = [cs // 16 for cs in CHUNK_SIZES]   # idx columns per chunk
CPC_OFF = [sum(CPCS[:i]) for i in range(N_CHUNKS + 1)]
NBLANK_IDS = 16
N_ITER = 8
BAND = N_ITER + 1                 # out[i] depends on e[i-8..i]
PSW = 512                         # psum bank width (f32)
NPC = D // PSW                    # psum column chunks per row


def _cdiv(a, b):
    return (a + b - 1) // b


def _compute_coeffs(x):
    """C[b, s, d] for d=0..8 (float64 holds small ints exactly)."""
    b, s = x.shape
    blank = ((x >= 0) & (x < NBLANK_IDS)).astype(np.float64)
    shift_r = lambda t: np.concatenate([np.zeros_like(t[:, :1]), t[:, :-1]], axis=1)
    first = np.maximum(blank - shift_r(blank), 0.0)
    m = np.concatenate([first[:, 1:], np.zeros_like(first[:, :1])], axis=1)
    C = np.zeros((b, s, BAND))
    C[:, :, 0] = 1.0
    for k in range(1, N_ITER + 1):
        m_k = np.zeros_like(m)
        m_k[:, k:] = m[:, :-k]
        Cs = np.zeros_like(C)
        Cs[:, 1:, 1:] = C[:, :-1, :-1]
        C = C + m_k[:, :, None] * Cs
    return C


def _wrap16(vals, ncols):
    """Wrap a 1-D index list into the [128, ncols] int16 layout the SWDGE
    gather/scatter ucode expects: slot j at [j % 16, j // 16], replicated
    across the eight 16-partition Q7 core groups."""
    blk = np.zeros((16, ncols), dtype=np.int16)
    v = np.asarray(vals, dtype=np.int16)
    for j in range(len(v)):
        blk[j % 16, j // 16] = v[j]
    return np.tile(blk, (8, 1))


def _prepare(x_np):
    """All host-side index/W prep. Returns per-core arrays + meta.

    The device table is emb[uniq] with one extra all-zero row at index NV:
    dead band-source slots index it so they contribute exact zeros.
    """
    uniq, inv = np.unique(x_np, return_inverse=True)
    ridx = inv.reshape(x_np.shape).astype(np.int64)
    NV = len(uniq)
    assert NV + 1 <= 32767, "int16 gather index overflow"
    ZROW = NV                                         # the appended zero row

    C = _compute_coeffs(x_np)
    aff = (C[:, :, 1:] != 0).any(axis=2)              # [B,S]

    cores = []
    for c in range(N_CORES):
        b, h = c // 2, c % 2
        s0 = h * RPC
        midx = np.zeros((128, CPC_OFF[-1]), dtype=np.int16)
        for ch in range(N_CHUNKS):
            cs, gpp = CHUNK_SIZES[ch], GPPS[ch]
            slots = np.empty(cs, dtype=np.int64)
            for j in range(cs):
                l = (j % 128) * gpp + (j // 128) + CHUNK_OFF[ch]
                slots[j] = ridx[b, s0 + l]
            midx[:, CPC_OFF[ch]:CPC_OFF[ch + 1]] = _wrap16(slots, CPCS[ch])

        # affected rows per region (region = run of chunks; the last region
        # is the final chunk alone so the tail scatter is tiny), split
        # further if rows would need more than one 128-row source tile
        rows_all = np.nonzero(aff[b, s0:s0 + RPC])[0]
        Cc = C[b, s0:s0 + RPC]                        # [RPC, 9]
        halves = []
        for lo, hi in REGIONS:
            r0, r1 = CHUNK_OFF[lo], CHUNK_OFF[hi]
            rh = rows_all[(rows_all >= r0) & (rows_all < r1)]
            parts = []
            cur, srcs = [], set()
            for r in rh:
                new = {int(r) - d for d in range(1, N_ITER + 1) if Cc[r, d] != 0}
                if len(cur) >= 128 or len(srcs | new) > 128:
                    parts.append(np.array(cur, dtype=np.int64))
                    cur, srcs = [], set()
                cur.append(int(r))
                srcs |= new
            if cur:
                parts.append(np.array(cur, dtype=np.int64))
            halves.append(parts)
        cores.append(dict(b=b, s0=s0, halves=halves, Cc=Cc, midx=midx))

    # group g of region h waits for the writebacks covering that region
    H = [max(len(co["halves"][h]) for co in cores) for h in range(len(REGIONS))]
    G = sum(H)
    meta = dict(NV=NV, G=G, wait_chunks=[])
    if G == 0:
        for co in cores:
            co.update(bidx=None, sidx=None, wmat=None, cnts=None)
        return uniq, cores, meta
    group_defs = []   # (region, index_within_region)
    for h in range(len(REGIONS)):
        for k in range(H[h]):
            group_defs.append((h, k))
            meta["wait_chunks"].append(list(range(*REGIONS[h])))
    for co in cores:
        co["rows_g"] = [co["halves"][h][k] if k < len(co["halves"][h])
                        else np.empty(0, dtype=np.int64)
                        for h, k in group_defs]

    for co in cores:
        b, s0, Cc = co["b"], co["s0"], co["Cc"]
        bidx = np.zeros((128, G * 8), dtype=np.int16)
        sidx = np.zeros((128, G * 8), dtype=np.int16)
        wmat = np.zeros((128, G * 128), dtype=np.float32)
        cnts = np.zeros((1, G), dtype=np.int32)       # scatter counts
        for g in range(G):
            rg = co["rows_g"][g]
            cnts[0, g] = max(len(rg), 1)
            srcs = sorted({int(r) - d for r in rg
                           for d in range(1, N_ITER + 1) if Cc[r, d] != 0})
            assert len(srcs) <= 128
            src_slot = {s: k for k, s in enumerate(srcs)}
            vals = np.full(128, ZROW, dtype=np.int64)
            for s, k in src_slot.items():
                vals[k] = ridx[b, s0 + s]
            bidx[:, g * 8:(g + 1) * 8] = _wrap16(vals, 8)
            for m, r in enumerate(rg):
                for d in range(1, N_ITER + 1):
                    if Cc[r, d] != 0:
                        wmat[src_slot[int(r) - d], g * 128 + m] = Cc[r, d]

            tgts = np.full(128, -1, dtype=np.int64)
            if len(rg):
                tgts[:len(rg)] = rg
            else:
                tgts[0] = 0   # adds an exact 0 to row 0
            sidx[:, g * 8:(g + 1) * 8] = _wrap16(tgts, 8)
        co.update(bidx=bidx, sidx=sidx, wmat=wmat, cnts=cnts)
    return uniq, cores, meta


def _build_program(NV, G, wait_chunks):
    import concourse.bacc as bacc
    import concourse.mybir as mybir
    from concourse.library_config import mlp

    f32, i16, i32 = mybir.dt.float32, mybir.dt.int16, mybir.dt.int32
    bf16 = mybir.dt.bfloat16

    nc = bacc.Bacc("TRN2", target_bir_lowering=False, debug=False,
                   enable_asserts=False, num_devices=N_CORES,
                   num_swdge_queues=4, dynamic_dma_scratch_size=65536)
    table = nc.dram_tensor("table", [NV + 1, D], bf16, kind="ExternalInput")
    midx_d = nc.dram_tensor("midx", [128, CPC_OFF[-1]], i16, kind="ExternalInput")
    out_d = nc.dram_tensor("out", [RPC, D], bf16, kind="ExternalOutput")
    if G:
        bidx_d = nc.dram_tensor("bidx", [128, G * 8], i16, kind="ExternalInput")
        sidx_d = nc.dram_tensor("sidx", [128, G * 8], i16, kind="ExternalInput")
        wmat_d = nc.dram_tensor("wmat", [128, G * 128], bf16, kind="ExternalInput")
        cnts_d = nc.dram_tensor("cnts", [1, G], i32, kind="ExternalInput")

    from contextlib import ExitStack
    with ExitStack() as st:
        # every chunk gets its own buffer (bf16 halves SBUF): no reuse waits
        mbuf = [st.enter_context(nc.sbuf_tensor(f"mbuf{i}", [128, GPPS[i], D], bf16))
                for i in range(N_CHUNKS)]
        midx_s = st.enter_context(nc.sbuf_tensor("midx_s", [128, CPC_OFF[-1]], i16))
        m_sem = st.enter_context(nc.semaphore("m_sem"))
        g_sems = [st.enter_context(nc.semaphore(f"g_sem{c}")) for c in range(N_CHUNKS)]
        w_sems = [st.enter_context(nc.semaphore(f"w_sem{c}")) for c in range(N_CHUNKS)]
        if G:
            etile = [st.enter_context(nc.sbuf_tensor(f"etile{g}", [128, 1, D], bf16))
                     for g in range(G)]
            deltas = [st.enter_context(
                nc.sbuf_tensor(f"delta{g}", [128, 1, D], bf16))
                for g in range(G)]
            npsets = min(G, 2)
            psum = [[st.enter_context(
                nc.psum_tensor(f"ps{e}_{c}", [128, PSW], f32))
                for c in range(NPC)] for e in range(npsets)]
            bidx_s = st.enter_context(nc.sbuf_tensor("bidx_s", [128, G * 8], i16))
            sidx_s = st.enter_context(nc.sbuf_tensor("sidx_s", [128, G * 8], i16))
            wmat_s = st.enter_context(nc.sbuf_tensor("wmat_s", [128, G * 128], bf16))
            cnts_s = st.enter_context(nc.sbuf_tensor("cnts_s", [1, G], i32))
            nregs = [st.enter_context(nc.gpsimd.register(f"nreg{g}"))
                     for g in range(G)]
            bi_sem = st.enter_context(nc.semaphore("bi_sem"))
            si_sem = st.enter_context(nc.semaphore("si_sem"))
            cd_sem = st.enter_context(nc.semaphore("cd_sem"))
            b_sems = [st.enter_context(nc.semaphore(f"b_sem{g}")) for g in range(G)]
            pe_sems = [st.enter_context(nc.semaphore(f"pe_sem{g}")) for g in range(G)]
            d_sems = [st.enter_context(nc.semaphore(f"d_sem{g}")) for g in range(G)]
            s_sem = st.enter_context(nc.semaphore("s_sem"))
            p_sem = st.enter_context(nc.semaphore("p_sem"))
        block = st.enter_context(nc.Block(no_gpsimd_drain=True))

        def writeback(eng, ch):
            eng.wait_ge(g_sems[ch], 16)
            dst = out_d[CHUNK_OFF[ch]:CHUNK_OFF[ch + 1], :].rearrange(
                "(p g) e -> p g e", g=GPPS[ch])
            eng.dma_start(dst, mbuf[ch][:, :, :]).then_inc(w_sems[ch], 16)

        @block.sync
        def _(sync):
            sync.dma_start(midx_s[:, :], midx_d[:, :]).then_inc(m_sem, 16)
            for ch in range(0, N_CHUNKS, 2):
                writeback(sync, ch)

        @block.scalar
        def _(scalar):
            if G:
                scalar.dma_start(bidx_s[:, :], bidx_d[:, :]).then_inc(bi_sem, 16)
                scalar.dma_start(cnts_s[:, :], cnts_d[:, :]).then_inc(bi_sem, 16)
                scalar.dma_start(wmat_s[:, :], wmat_d[:, :]).then_inc(cd_sem, 16)
                scalar.dma_start(sidx_s[:, :], sidx_d[:, :]).then_inc(si_sem, 16)
            for ch in range(1, N_CHUNKS, 2):
                writeback(scalar, ch)

        @block.gpsimd
        def _(gp):
            gp.load_library(mlp)
            gp.wait_ge(m_sem, 16)

            def main_gather(ch):
                cs = CHUNK_SIZES[ch]
                gp.dma_gather(mbuf[ch][:, :, :], table[:, :],
                              midx_s[:, CPC_OFF[ch]:CPC_OFF[ch + 1]],
                              cs, cs, D,
                              single_packet=False).then_inc(g_sems[ch], 16)

            def band_gather(g):
                gp.dma_gather(etile[g][:, :, :], table[:, :],
                              bidx_s[:, g * 8:(g + 1) * 8],
                              128, 128, D,
                              single_packet=False,
                              queue_num=1 + g % 2).then_inc(b_sems[g], 16)

            main_gather(0)
            main_gather(1)
            if G:
                gp.wait_ge(bi_sem, 32)
                for g in range(G):
                    band_gather(g)
            for ch in range(2, N_CHUNKS):
                main_gather(ch)
            if G:
                # scatters: descriptors prepped early (reads sidx + count
                # regs now), fired by trigger_dma once the writebacks and
                # deltas land - keeps desc-gen off the critical tail
                gp.wait_ge(si_sem, 16)
                for g in range(G):
                    gp.reg_load(nregs[g], cnts_s[0:1, g:g + 1])
                    gp.dma_scatter_add(out_d[:, :], deltas[g][:, :, :],
                                       sidx_s[:, g * 8:(g + 1) * 8],
                                       128, nregs[g], D,
                                       single_packet=False,
                                       queue_num=3, prepare_only=True,
                                       sem=s_sem).then_inc(p_sem, 1)
                for g in range(G):
                    gp.wait_ge(p_sem, g + 1)
                    for c in wait_chunks[g]:
                        gp.wait_ge(w_sems[c], 16)
                    gp.wait_ge(d_sems[g], 1)
                    gp.trigger_dma(count=1, queue_num=3)
                gp.wait_ge(s_sem, 16 * G)

        @block.tensor
        def _(pe):
            if not G:
                return
            pe.wait_ge(cd_sem, 16)
            for g in range(G):
                e = g % 2
                if g >= 2:
                    pe.wait_ge(d_sems[g - 2], 1)   # psum set free again
                pe.wait_ge(b_sems[g], 16)
                for c in range(NPC):
                    ins = pe.matmul(
                        psum[e][c][:, :],
                        wmat_s[:, g * 128:(g + 1) * 128],
                        etile[g][:, 0, c * PSW:(c + 1) * PSW],
                        start=True, stop=True)
                ins.then_inc(pe_sems[g], 1)

        @block.vector
        def _(v):
            if not G:
                return
            for g in range(G):
                v.wait_ge(pe_sems[g], 1)
                for c in range(NPC):
                    ins = v.tensor_copy(deltas[g][:, 0, c * PSW:(c + 1) * PSW],
                                        psum[g % 2][c][:, :])
                ins.then_inc(d_sems[g], 1)

    nc.compile()
    return nc


_CACHE = {}
_LAST_RESULT = None


def kernel(x, emb_table):
    global _LAST_RESULT
    import ml_dtypes
    from concourse.bass_utils import run_bass_kernel_spmd

    x_np = np.asarray(x)
    emb_np = np.asarray(emb_table, dtype=np.float32)
    uniq, cores, meta = _prepare(x_np)
    table_sl = np.zeros((meta["NV"] + 1, D), dtype=ml_dtypes.bfloat16)
    table_sl[:meta["NV"]] = emb_np[uniq].astype(ml_dtypes.bfloat16)

    key = (meta["NV"], meta["G"],
           tuple(map(tuple, meta["wait_chunks"])))
    if key not in _CACHE:
        _CACHE[key] = _build_program(meta["NV"], meta["G"],
                                     meta["wait_chunks"])
    nc = _CACHE[key]

    in_maps = []
    for co in cores:
        m = {"table": table_sl, "midx": co["midx"]}
        if meta["G"]:
            m.update(bidx=co["bidx"], sidx=co["sidx"],
                     wmat=co["wmat"].astype(ml_dtypes.bfloat16),
                     cnts=co["cnts"])
        in_maps.append(m)

    res = run_bass_kernel_spmd(nc, in_maps, core_ids=list(range(N_CORES)))
    _LAST_RESULT = res
    full = np.empty((B, S, D), dtype=np.float32)
    for c in range(N_CORES):
        b, h = c // 2, c % 2
        full[b, h * RPC:(h + 1) * RPC, :] = res.results[c]["out"].astype(np.float32)
    return full
